# revision 6
# baseline (speedup 1.0000x reference)
"""Trainium2 Bass kernel for nn_DiffModule_40827959116531 (sparse_attention).

Reference computation (per batch element b):
    sv  = src @ W1 + b1                      # (L, O)
    tk  = trg @ W2 + b2                      # (N, O)
    tv  = trg @ W1 + b1                      # (N, O)
    score = sv @ tk.T / sqrt(O)              # (L, N)
    prob  = softmax(score, axis=-1)
    ctx   = prob @ tv                        # (L, O)
    h   = concat([sv, sv - ctx], -1)         # (L, 2O)
    h2  = relu(h @ W3a + b3a)                # (L, O)
    out = h2 @ W3b + b3b                     # (L, O)

Sharding: data-parallel over B=32 across 8 cores (4 batch elements per
core); weights replicated.

Per-core dataflow (all matmuls in bf16 with fp32 PSUM accumulation):
  - src/trg are loaded row-major, cast to bf16, and transposed on-chip
    (DMA XBAR transpose) into S = src^T, T = trg^T with the contraction
    dim D on partitions.
  - sv_T [O-part, L]   = matmul(lhsT=W1, rhs=S)   (+ b1 per-partition)
  - tk_T [O-part, N]   = matmul(lhsT=W2, rhs=T)   (+ b2 per-partition)
  - tv   [N-part, O]   = matmul(lhsT=T,  rhs=W1)  (+ b1 via K=1 ones-row matmul)
  - score_T [N-part, L] = matmul(lhsT=tk_T, rhs=sv_T); e = exp(score/32)
    (softmax max-subtraction skipped: scores have std ~0.33, exp is safe)
  - denom over the partition dim via ones-column matmul; r = 1/denom is
    broadcast across partitions with a K=1 fp32 matmul.
  - ctx_T [O-part, L] = matmul(lhsT=tv, rhs=e); hl = sv_T - ctx_T * r
  - h2_T [O-part, L]  = relu(matmul(lhsT=W3a, rhs=[sv_T; hl]) + b3a)
  - out  [L-part, O]  = matmul(lhsT=h2_T, rhs=W3b) (+ b3b via K=1 matmul)
    -> written to DRAM in natural row-major layout.
"""

import math
from contextlib import ExitStack

import numpy as np

import concourse.bass as bass
import concourse.mybir as mybir
import concourse.tile as tile
from concourse import bacc
from concourse.bass_utils import run_bass_kernel_spmd

P = 128
B_FULL = 32
N_CORES = 8
BS = B_FULL // N_CORES  # 4 batch elements per core
L = 1024
N = 1024
D = 1024
O = 1024

F32 = mybir.dt.float32
BF16 = mybir.dt.bfloat16
AF = mybir.ActivationFunctionType

LC = 512          # l-chunk size for phase B
N_LC = L // LC    # 2 chunks
KD = D // P       # 8 contraction tiles over D
KO = O // P       # 8 tiles over O
KN = N // P       # 8 tiles over N
K2O = 2 * O // P  # 16 tiles over 2O

INV_SQRT_O = 1.0 / math.sqrt(O)


def _load_weight_bf16(nc, sb_stage, dst, w_dram, ktiles):
    """DMA a (ktiles*128, 1024) f32 weight into [128, ktiles, 1024] bf16."""
    for k in range(ktiles):
        stg = sb_stage.tile([P, 1024], F32, tag="stage")
        nc.sync.dma_start(stg[:], w_dram.ap()[k * P:(k + 1) * P, :])
        nc.vector.tensor_copy(dst[:, k, :], stg[:])


def _build(nc, tc):
    src = nc.dram_tensor("src", [BS, L, D], F32, kind="ExternalInput")
    trg = nc.dram_tensor("trg", [BS, N, D], F32, kind="ExternalInput")
    w1 = nc.dram_tensor("W1", [D, O], F32, kind="ExternalInput")
    b1 = nc.dram_tensor("b1", [O], F32, kind="ExternalInput")
    w2 = nc.dram_tensor("W2", [D, O], F32, kind="ExternalInput")
    b2 = nc.dram_tensor("b2", [O], F32, kind="ExternalInput")
    w3a = nc.dram_tensor("W3a", [2 * O, O], F32, kind="ExternalInput")
    b3a = nc.dram_tensor("b3a", [O], F32, kind="ExternalInput")
    w3b = nc.dram_tensor("W3b", [O, O], F32, kind="ExternalInput")
    b3b = nc.dram_tensor("b3b", [O], F32, kind="ExternalInput")
    out = nc.dram_tensor("out", [BS, L, O], F32, kind="ExternalOutput")

    ctx = ExitStack()
    singles = ctx.enter_context(tc.tile_pool(name="singles", bufs=1))
    stage = ctx.enter_context(tc.tile_pool(name="stage", bufs=2))
    natbf = ctx.enter_context(tc.tile_pool(name="natbf", bufs=2))
    stp = ctx.enter_context(tc.tile_pool(name="stp", bufs=2))
    actp = ctx.enter_context(tc.tile_pool(name="actp", bufs=1))
    ehh = ctx.enter_context(tc.tile_pool(name="ehh", bufs=2))
    smallp = ctx.enter_context(tc.tile_pool(name="smallp", bufs=2))
    outp = ctx.enter_context(tc.tile_pool(name="outp", bufs=2))
    psum = ctx.enter_context(tc.tile_pool(name="psum", bufs=6, space="PSUM"))
    auxps = ctx.enter_context(tc.tile_pool(name="auxps", bufs=1, space="PSUM"))

    # ---- constants: weights (bf16), biases, ones ----
    w1bf = singles.tile([P, KD, O], BF16)
    w2bf = singles.tile([P, KD, O], BF16)
    w3abf = singles.tile([P, K2O, O], BF16)
    w3bbf = singles.tile([P, KO, O], BF16)
    _load_weight_bf16(nc, stage, w1bf, w1, KD)
    _load_weight_bf16(nc, stage, w2bf, w2, KD)
    _load_weight_bf16(nc, stage, w3abf, w3a, K2O)
    _load_weight_bf16(nc, stage, w3bbf, w3b, KO)

    # per-partition bias columns [128, 8] f32: col j partition p = bias[j*128+p]
    b1col = singles.tile([P, KO], F32)
    b2col = singles.tile([P, KO], F32)
    b3acol = singles.tile([P, KO], F32)
    nc.sync.dma_start(b1col[:], b1.ap().rearrange("(oo op) -> op oo", op=P))
    nc.sync.dma_start(b2col[:], b2.ap().rearrange("(oo op) -> op oo", op=P))
    nc.sync.dma_start(b3acol[:], b3a.ap().rearrange("(oo op) -> op oo", op=P))

    # bias rows [1, 1024] bf16 for K=1 matmul bias adds (b1 for tv, b3b for out)
    b1row = singles.tile([1, O], BF16)
    b3brow = singles.tile([1, O], BF16)
    for row_dram, row_bf in ((b1, b1row), (b3b, b3brow)):
        rowtmp = smallp.tile([1, O], F32, tag="rowtmp", bufs=1)
        nc.sync.dma_start(rowtmp[:], row_dram.ap()[None, :])
        nc.vector.tensor_copy(row_bf[:], rowtmp[:])

    ones_col = singles.tile([P, 1], BF16)   # lhsT for partition-sum (denominator)
    nc.vector.memset(ones_col[:], 1.0)
    ones_row_bf = singles.tile([1, P], BF16)  # lhsT for K=1 bias-row matmuls
    nc.vector.memset(ones_row_bf[:], 1.0)
    ones_row_f = singles.tile([1, P], F32)    # lhsT for K=1 fp32 broadcast matmul
    nc.vector.memset(ones_row_f[:], 1.0)

    for b in range(BS):
        # ---- load + transpose src/trg (bf16) ----
        s_t = stp.tile([P, KD, L], BF16, tag="st")  # src^T: [d-part, d-tile, l]
        t_t = stp.tile([P, KD, N], BF16, tag="st")  # trg^T
        for dram, dest, rows in ((src, s_t, L), (trg, t_t, N)):
            for lo in range(rows // P):
                stg = stage.tile([P, D], F32, tag="stage")
                nc.sync.dma_start(stg[:], dram.ap()[b, lo * P:(lo + 1) * P, :])
                nat = natbf.tile([P, D], BF16, tag="natbf")
                nc.vector.tensor_copy(nat[:], stg[:])
                nc.sync.dma_start_transpose(dest[:, :, lo * P:(lo + 1) * P], nat[:])

        # ---- phase A: sv_T, tk_T, tv ----
        svt = actp.tile([P, KO, L], BF16, tag="svt")
        tkt = actp.tile([P, KO, N], BF16, tag="tkt")
        tv = actp.tile([P, KN, O], BF16, tag="tv")
        for j in range(KO):
            for lc in range(N_LC):
                ps = psum.tile([P, LC], F32)
                for k in range(KD):
                    nc.tensor.matmul(
                        ps[:], w1bf[:, k, j * P:(j + 1) * P],
                        s_t[:, k, lc * LC:(lc + 1) * LC],
                        start=(k == 0), stop=(k == KD - 1))
                nc.scalar.activation(
                    svt[:, j, lc * LC:(lc + 1) * LC], ps[:], AF.Identity,
                    bias=b1col[:, j:j + 1])
        for j in range(KO):
            for nch in range(N // LC):
                ps = psum.tile([P, LC], F32)
                for k in range(KD):
                    nc.tensor.matmul(
                        ps[:], w2bf[:, k, j * P:(j + 1) * P],
                        t_t[:, k, nch * LC:(nch + 1) * LC],
                        start=(k == 0), stop=(k == KD - 1))
                nc.scalar.activation(
                    tkt[:, j, nch * LC:(nch + 1) * LC], ps[:], AF.Identity,
                    bias=b2col[:, j:j + 1])
        for i in range(KN):
            for oc in range(O // LC):
                ps = psum.tile([P, LC], F32)
                for k in range(KD):
                    nc.tensor.matmul(
                        ps[:], t_t[:, k, i * P:(i + 1) * P],
                        w1bf[:, k, oc * LC:(oc + 1) * LC],
                        start=(k == 0), stop=False)
                nc.tensor.matmul(
                    ps[:], ones_row_bf[:1, :], b1row[:1, oc * LC:(oc + 1) * LC],
                    start=False, stop=True)
                nc.scalar.activation(tv[:, i, oc * LC:(oc + 1) * LC], ps[:], AF.Copy)

        # ---- phase B: per l-chunk ----
        for lc in range(N_LC):
            lsl = slice(lc * LC, (lc + 1) * LC)
            # score_T -> e = exp(score / sqrt(O)), [n-part, l]
            e = ehh.tile([P, KN, LC], BF16, tag="ehh")
            for i in range(KN):
                ps = psum.tile([P, LC], F32)
                for k in range(KO):
                    nc.tensor.matmul(
                        ps[:], tkt[:, k, i * P:(i + 1) * P], svt[:, k, lsl],
                        start=(k == 0), stop=(k == KO - 1))
                nc.scalar.activation(e[:, i, :], ps[:], AF.Exp, scale=INV_SQRT_O)
            # denominator: sum e over n (partitions) via ones-column matmul
            d_ps = auxps.tile([1, LC], F32, tag="dps")
            for i in range(KN):
                nc.tensor.matmul(d_ps[:], ones_col[:, :1], e[:, i, :],
                                 start=(i == 0), stop=(i == KN - 1))
            r_sb = smallp.tile([1, LC], F32, tag="rsb", bufs=1)
            nc.vector.reciprocal(r_sb[:], d_ps[:])
            # broadcast r over 128 partitions with a K=1 fp32 matmul
            r_ps = auxps.tile([P, LC], F32, tag="rps")
            nc.tensor.matmul(r_ps[:], ones_row_f[:1, :], r_sb[:1, :],
                             start=True, stop=True)
            rbc = smallp.tile([P, LC], F32, tag="rbc", bufs=1)
            nc.vector.tensor_copy(rbc[:], r_ps[:])
            # ctx_T + normalize + hl = sv_T - ctx_T/denom
            hl = ehh.tile([P, KO, LC], BF16, tag="ehh")
            for j in range(KO):
                ps = psum.tile([P, LC], F32)
                for i in range(KN):
                    nc.tensor.matmul(
                        ps[:], tv[:, i, j * P:(j + 1) * P], e[:, i, :],
                        start=(i == 0), stop=(i == KN - 1))
                ctxn = smallp.tile([P, LC], F32, tag="ctxn", bufs=1)
                nc.vector.tensor_mul(ctxn[:], ps[:], rbc[:])
                nc.vector.tensor_sub(hl[:, j, :], svt[:, j, lsl], ctxn[:])
            # fc3a: h2 = relu([sv_T; hl] contracted with W3a + b3a)
            h2 = ehh.tile([P, KO, LC], BF16, tag="ehh")
            for j2 in range(KO):
                ps = psum.tile([P, LC], F32)
                for k in range(K2O):
                    rhs = svt[:, k, lsl] if k < KO else hl[:, k - KO, :]
                    nc.tensor.matmul(
                        ps[:], w3abf[:, k, j2 * P:(j2 + 1) * P], rhs,
                        start=(k == 0), stop=(k == K2O - 1))
                nc.scalar.activation(h2[:, j2, :], ps[:], AF.Relu,
                                     bias=b3acol[:, j2:j2 + 1])
            # fc3b: out natural [l-part, o] + b3b via K=1 matmul
            for lt in range(LC // P):
                for oc in range(O // LC):
                    ps = psum.tile([P, LC], F32)
                    for k in range(KO):
                        nc.tensor.matmul(
                            ps[:], h2[:, k, lt * P:(lt + 1) * P],
                            w3bbf[:, k, oc * LC:(oc + 1) * LC],
                            start=(k == 0), stop=False)
                    nc.tensor.matmul(
                        ps[:], ones_row_bf[:1, :], b3brow[:1, oc * LC:(oc + 1) * LC],
                        start=False, stop=True)
                    o_sb = outp.tile([P, LC], F32, tag="osb")
                    nc.scalar.activation(o_sb[:], ps[:], AF.Copy)
                    nc.sync.dma_start(
                        out.ap()[b, lc * LC + lt * P: lc * LC + (lt + 1) * P,
                                 oc * LC:(oc + 1) * LC],
                        o_sb[:])

    ctx.close()


_NC_CACHE = None


def _get_nc():
    global _NC_CACHE
    if _NC_CACHE is None:
        nc = bacc.Bacc("TRN2", target_bir_lowering=False, debug=False,
                       num_devices=N_CORES)
        with tile.TileContext(nc) as tc:
            _build(nc, tc)
        nc.compile()
        _NC_CACHE = nc
    return _NC_CACHE


def kernel(**inputs):
    nc = _get_nc()
    src = np.ascontiguousarray(inputs["src"], dtype=np.float32)
    trg = np.ascontiguousarray(inputs["trg"], dtype=np.float32)
    shared = {
        k: np.ascontiguousarray(np.asarray(inputs[k], dtype=np.float32))
        for k in ("W1", "b1", "W2", "b2", "W3a", "b3a", "W3b", "b3b")
    }
    in_maps = []
    for c in range(N_CORES):
        m = dict(shared)
        m["src"] = src[c * BS:(c + 1) * BS]
        m["trg"] = trg[c * BS:(c + 1) * BS]
        in_maps.append(m)
    res = run_bass_kernel_spmd(nc, in_maps, core_ids=list(range(N_CORES)))
    return np.concatenate([r["out"] for r in res.results], axis=0)


# revision 7
# speedup vs baseline: 1.2222x; 1.2222x over previous
"""Trainium2 Bass kernel for nn_DiffModule_40827959116531 (sparse_attention).

Reference computation (per batch element b):
    sv  = src @ W1 + b1                      # (L, O)
    tk  = trg @ W2 + b2                      # (N, O)
    tv  = trg @ W1 + b1                      # (N, O)
    score = sv @ tk.T / sqrt(O)              # (L, N)
    prob  = softmax(score, axis=-1)
    ctx   = prob @ tv                        # (L, O)
    h   = concat([sv, sv - ctx], -1)         # (L, 2O)
    h2  = relu(h @ W3a + b3a)                # (L, O)
    out = h2 @ W3b + b3b                     # (L, O)

Sharding: data-parallel over B=32 across 8 cores (4 batch elements per
core); weights replicated. Host-side marshalling casts activations and
weights to bf16 and pre-transposes src/trg to [D, L] so the contraction
dim lands on SBUF partitions with plain contiguous DMAs.

Per-core dataflow (bf16 operands, fp32 PSUM accumulation):
  - sv_T [O-part, L]   = matmul(lhsT=W1, rhs=srcT)  (+ b1 per-partition)
  - tk_T [O-part, N]   = matmul(lhsT=W2, rhs=trgT)  (+ b2 per-partition)
  - tv   [N-part, O]   = matmul(lhsT=trgT, rhs=W1)  (+ b1 via K=1 ones-row matmul)
  - score_T [N-part, L] = matmul(lhsT=tk_T, rhs=sv_T); e = exp(score/32)
    (softmax max-subtraction skipped: scores have std ~0.33, exp is safe)
  - denom over the partition dim via ones-column matmul; r = 1/denom is
    broadcast across partitions with a K=1 fp32 matmul.
  - ctx_T [O-part, L] = matmul(lhsT=tv, rhs=e); hl = sv_T - ctx_T * r
  - h2_T [O-part, L]  = relu(matmul(lhsT=W3a, rhs=[sv_T; hl]) + b3a)
  - out  [L-part, O]  = matmul(lhsT=h2_T, rhs=W3b) (+ b3b via K=1 matmul)
    -> written to DRAM in natural row-major layout (fp32).
"""

import math
from contextlib import ExitStack

import ml_dtypes
import numpy as np

import concourse.bass as bass
import concourse.mybir as mybir
import concourse.tile as tile
from concourse import bacc
from concourse.bass_utils import run_bass_kernel_spmd

P = 128
B_FULL = 32
N_CORES = 8
BS = B_FULL // N_CORES  # 4 batch elements per core
L = 1024
N = 1024
D = 1024
O = 1024

F32 = mybir.dt.float32
BF16 = mybir.dt.bfloat16
AF = mybir.ActivationFunctionType
NP_BF16 = ml_dtypes.bfloat16

LC = 512          # l-chunk size for phase B
N_LC = L // LC    # 2 chunks
KD = D // P       # 8 contraction tiles over D
KO = O // P       # 8 tiles over O
KN = N // P       # 8 tiles over N
K2O = 2 * O // P  # 16 tiles over 2O

INV_SQRT_O = 1.0 / math.sqrt(O)


def _load_weight(nc, dst, w_dram, ktiles):
    """DMA a (ktiles*128, 1024) bf16 weight into [128, ktiles, 1024]."""
    for k in range(ktiles):
        nc.sync.dma_start(dst[:, k, :], w_dram.ap()[k * P:(k + 1) * P, :])


def _load_st(nc, dest, dram, b):
    """DMA a pre-transposed (D, rows) bf16 activation into [128, KD, rows]."""
    for k in range(KD):
        nc.sync.dma_start(dest[:, k, :], dram.ap()[b, k * P:(k + 1) * P, :])


def _build(nc, tc):
    src_t = nc.dram_tensor("srcT", [BS, D, L], BF16, kind="ExternalInput")
    trg_t = nc.dram_tensor("trgT", [BS, D, N], BF16, kind="ExternalInput")
    w1 = nc.dram_tensor("W1bf", [D, O], BF16, kind="ExternalInput")
    w2 = nc.dram_tensor("W2bf", [D, O], BF16, kind="ExternalInput")
    w3a = nc.dram_tensor("W3abf", [2 * O, O], BF16, kind="ExternalInput")
    w3b = nc.dram_tensor("W3bbf", [O, O], BF16, kind="ExternalInput")
    b1 = nc.dram_tensor("b1", [O], F32, kind="ExternalInput")
    b2 = nc.dram_tensor("b2", [O], F32, kind="ExternalInput")
    b3a = nc.dram_tensor("b3a", [O], F32, kind="ExternalInput")
    b1bf = nc.dram_tensor("b1bf", [O], BF16, kind="ExternalInput")
    b3bbf = nc.dram_tensor("b3bbf", [O], BF16, kind="ExternalInput")
    out = nc.dram_tensor("out", [BS, L, O], F32, kind="ExternalOutput")

    ctx = ExitStack()
    singles = ctx.enter_context(tc.tile_pool(name="singles", bufs=1))
    stp = ctx.enter_context(tc.tile_pool(name="stp", bufs=2))
    actp = ctx.enter_context(tc.tile_pool(name="actp", bufs=1))
    ehh = ctx.enter_context(tc.tile_pool(name="ehh", bufs=3))
    smallp = ctx.enter_context(tc.tile_pool(name="smallp", bufs=2))
    outp = ctx.enter_context(tc.tile_pool(name="outp", bufs=3))
    psum = ctx.enter_context(tc.tile_pool(name="psum", bufs=6, space="PSUM"))
    auxps = ctx.enter_context(tc.tile_pool(name="auxps", bufs=1, space="PSUM"))

    # ---- constants ----
    w1bf = singles.tile([P, KD, O], BF16)
    w2bf = singles.tile([P, KD, O], BF16)
    w3abf = singles.tile([P, K2O, O], BF16)
    w3bbf = singles.tile([P, KO, O], BF16)
    b1col = singles.tile([P, KO], F32)
    b2col = singles.tile([P, KO], F32)
    b3acol = singles.tile([P, KO], F32)
    b1row = singles.tile([1, O], BF16)
    b3brow = singles.tile([1, O], BF16)
    ones_col = singles.tile([P, 1], BF16)
    ones_row_bf = singles.tile([1, P], BF16)
    ones_row_f = singles.tile([1, P], F32)

    # W1 first: the first matmul group only needs W1 + srcT(b0).
    _load_weight(nc, w1bf, w1, KD)
    # batch 0 activations right behind W1 so phase A can start ASAP
    s_t0 = stp.tile([P, KD, L], BF16, tag="st")
    t_t0 = stp.tile([P, KD, N], BF16, tag="st")
    _load_st(nc, s_t0, src_t, 0)
    _load_st(nc, t_t0, trg_t, 0)
    # remaining weights + constants
    _load_weight(nc, w2bf, w2, KD)
    _load_weight(nc, w3abf, w3a, K2O)
    _load_weight(nc, w3bbf, w3b, KO)
    nc.sync.dma_start(b1col[:], b1.ap().rearrange("(oo op) -> op oo", op=P))
    nc.sync.dma_start(b2col[:], b2.ap().rearrange("(oo op) -> op oo", op=P))
    nc.sync.dma_start(b3acol[:], b3a.ap().rearrange("(oo op) -> op oo", op=P))
    nc.sync.dma_start(b1row[:], b1bf.ap()[None, :])
    nc.sync.dma_start(b3brow[:], b3bbf.ap()[None, :])
    nc.vector.memset(ones_col[:], 1.0)
    nc.vector.memset(ones_row_bf[:], 1.0)
    nc.vector.memset(ones_row_f[:], 1.0)

    for b in range(BS):
        if b == 0:
            s_t, t_t = s_t0, t_t0
        else:
            s_t = stp.tile([P, KD, L], BF16, tag="st")
            t_t = stp.tile([P, KD, N], BF16, tag="st")
            _load_st(nc, s_t, src_t, b)
            _load_st(nc, t_t, trg_t, b)

        # ---- phase A: sv_T, tk_T (ACT drain + bias), tv (DVE drain) ----
        svt = actp.tile([P, KO, L], BF16, tag="svt")
        tkt = actp.tile([P, KO, N], BF16, tag="tkt")
        tv = actp.tile([P, KN, O], BF16, tag="tv")
        for j in range(KO):
            for lc in range(N_LC):
                ps = psum.tile([P, LC], F32)
                for k in range(KD):
                    nc.tensor.matmul(
                        ps[:], w1bf[:, k, j * P:(j + 1) * P],
                        s_t[:, k, lc * LC:(lc + 1) * LC],
                        start=(k == 0), stop=(k == KD - 1))
                nc.scalar.activation(
                    svt[:, j, lc * LC:(lc + 1) * LC], ps[:], AF.Identity,
                    bias=b1col[:, j:j + 1])
        for j in range(KO):
            for nch in range(N // LC):
                ps = psum.tile([P, LC], F32)
                for k in range(KD):
                    nc.tensor.matmul(
                        ps[:], w2bf[:, k, j * P:(j + 1) * P],
                        t_t[:, k, nch * LC:(nch + 1) * LC],
                        start=(k == 0), stop=(k == KD - 1))
                nc.scalar.activation(
                    tkt[:, j, nch * LC:(nch + 1) * LC], ps[:], AF.Identity,
                    bias=b2col[:, j:j + 1])
        for i in range(KN):
            for oc in range(O // LC):
                ps = psum.tile([P, LC], F32)
                for k in range(KD):
                    nc.tensor.matmul(
                        ps[:], t_t[:, k, i * P:(i + 1) * P],
                        w1bf[:, k, oc * LC:(oc + 1) * LC],
                        start=(k == 0), stop=False)
                nc.tensor.matmul(
                    ps[:], ones_row_bf[:1, :], b1row[:1, oc * LC:(oc + 1) * LC],
                    start=False, stop=True)
                nc.vector.tensor_copy(tv[:, i, oc * LC:(oc + 1) * LC], ps[:])

        # ---- phase B: per l-chunk ----
        for lc in range(N_LC):
            lsl = slice(lc * LC, (lc + 1) * LC)
            # score_T -> e = exp(score / sqrt(O)), [n-part, l]
            e = ehh.tile([P, KN, LC], BF16, tag="ehh")
            for i in range(KN):
                ps = psum.tile([P, LC], F32)
                for k in range(KO):
                    nc.tensor.matmul(
                        ps[:], tkt[:, k, i * P:(i + 1) * P], svt[:, k, lsl],
                        start=(k == 0), stop=(k == KO - 1))
                nc.scalar.activation(e[:, i, :], ps[:], AF.Exp, scale=INV_SQRT_O)
            # denominator: sum e over n (partitions) via ones-column matmul
            d_ps = auxps.tile([1, LC], F32, tag="dps")
            for i in range(KN):
                nc.tensor.matmul(d_ps[:], ones_col[:, :1], e[:, i, :],
                                 start=(i == 0), stop=(i == KN - 1))
            r_sb = smallp.tile([1, LC], F32, tag="rsb", bufs=1)
            nc.vector.reciprocal(r_sb[:], d_ps[:])
            # broadcast r over 128 partitions with a K=1 fp32 matmul
            r_ps = auxps.tile([P, LC], F32, tag="rps")
            nc.tensor.matmul(r_ps[:], ones_row_f[:1, :], r_sb[:1, :],
                             start=True, stop=True)
            rbc = smallp.tile([P, LC], F32, tag="rbc", bufs=1)
            nc.vector.tensor_copy(rbc[:], r_ps[:])
            # ctx_T + normalize + hl = sv_T - ctx_T/denom
            hl = ehh.tile([P, KO, LC], BF16, tag="ehh")
            for j in range(KO):
                ps = psum.tile([P, LC], F32)
                for i in range(KN):
                    nc.tensor.matmul(
                        ps[:], tv[:, i, j * P:(j + 1) * P], e[:, i, :],
                        start=(i == 0), stop=(i == KN - 1))
                ctxn = smallp.tile([P, LC], F32, tag="ctxn", bufs=1)
                nc.vector.tensor_mul(ctxn[:], ps[:], rbc[:])
                nc.vector.tensor_sub(hl[:, j, :], svt[:, j, lsl], ctxn[:])
            # fc3a: h2 = relu([sv_T; hl] contracted with W3a + b3a)
            h2 = ehh.tile([P, KO, LC], BF16, tag="ehh")
            for j2 in range(KO):
                ps = psum.tile([P, LC], F32)
                for k in range(K2O):
                    rhs = svt[:, k, lsl] if k < KO else hl[:, k - KO, :]
                    nc.tensor.matmul(
                        ps[:], w3abf[:, k, j2 * P:(j2 + 1) * P], rhs,
                        start=(k == 0), stop=(k == K2O - 1))
                nc.scalar.activation(h2[:, j2, :], ps[:], AF.Relu,
                                     bias=b3acol[:, j2:j2 + 1])
            # fc3b: out natural [l-part, o] + b3b via K=1 matmul
            for lt in range(LC // P):
                for oc in range(O // LC):
                    ps = psum.tile([P, LC], F32)
                    for k in range(KO):
                        nc.tensor.matmul(
                            ps[:], h2[:, k, lt * P:(lt + 1) * P],
                            w3bbf[:, k, oc * LC:(oc + 1) * LC],
                            start=(k == 0), stop=False)
                    nc.tensor.matmul(
                        ps[:], ones_row_bf[:1, :], b3brow[:1, oc * LC:(oc + 1) * LC],
                        start=False, stop=True)
                    o_sb = outp.tile([P, LC], F32, tag="osb")
                    nc.vector.tensor_copy(o_sb[:], ps[:])
                    nc.sync.dma_start(
                        out.ap()[b, lc * LC + lt * P: lc * LC + (lt + 1) * P,
                                 oc * LC:(oc + 1) * LC],
                        o_sb[:])

    ctx.close()


_NC_CACHE = None


def _get_nc():
    global _NC_CACHE
    if _NC_CACHE is None:
        nc = bacc.Bacc("TRN2", target_bir_lowering=False, debug=False,
                       num_devices=N_CORES)
        with tile.TileContext(nc) as tc:
            _build(nc, tc)
        nc.compile()
        _NC_CACHE = nc
    return _NC_CACHE


def kernel(**inputs):
    nc = _get_nc()
    src = np.asarray(inputs["src"], dtype=np.float32)
    trg = np.asarray(inputs["trg"], dtype=np.float32)
    # host-side marshalling: bf16 cast + transpose so the contraction dim
    # (D) lands on SBUF partitions with contiguous DMAs on-device.
    src_t = np.ascontiguousarray(
        src.astype(NP_BF16).transpose(0, 2, 1))   # (B, D, L)
    trg_t = np.ascontiguousarray(
        trg.astype(NP_BF16).transpose(0, 2, 1))   # (B, D, N)
    shared = {
        "W1bf": np.ascontiguousarray(np.asarray(inputs["W1"], np.float32).astype(NP_BF16)),
        "W2bf": np.ascontiguousarray(np.asarray(inputs["W2"], np.float32).astype(NP_BF16)),
        "W3abf": np.ascontiguousarray(np.asarray(inputs["W3a"], np.float32).astype(NP_BF16)),
        "W3bbf": np.ascontiguousarray(np.asarray(inputs["W3b"], np.float32).astype(NP_BF16)),
        "b1": np.ascontiguousarray(np.asarray(inputs["b1"], np.float32)),
        "b2": np.ascontiguousarray(np.asarray(inputs["b2"], np.float32)),
        "b3a": np.ascontiguousarray(np.asarray(inputs["b3a"], np.float32)),
        "b1bf": np.ascontiguousarray(np.asarray(inputs["b1"], np.float32).astype(NP_BF16)),
        "b3bbf": np.ascontiguousarray(np.asarray(inputs["b3b"], np.float32).astype(NP_BF16)),
    }
    in_maps = []
    for c in range(N_CORES):
        m = dict(shared)
        m["srcT"] = src_t[c * BS:(c + 1) * BS]
        m["trgT"] = trg_t[c * BS:(c + 1) * BS]
        in_maps.append(m)
    res = run_bass_kernel_spmd(nc, in_maps, core_ids=list(range(N_CORES)))
    return np.concatenate([r["out"] for r in res.results], axis=0)


# revision 9
# speedup vs baseline: 1.3152x; 1.0761x over previous
"""Trainium2 Bass kernel for nn_DiffModule_40827959116531 (sparse_attention).

Reference computation (per batch element b):
    sv  = src @ W1 + b1                      # (L, O)
    tk  = trg @ W2 + b2                      # (N, O)
    tv  = trg @ W1 + b1                      # (N, O)
    score = sv @ tk.T / sqrt(O)              # (L, N)
    prob  = softmax(score, axis=-1)
    ctx   = prob @ tv                        # (L, O)
    h   = concat([sv, sv - ctx], -1)         # (L, 2O)
    h2  = relu(h @ W3a + b3a)                # (L, O)
    out = h2 @ W3b + b3b                     # (L, O)

Sharding: data-parallel over B=32 across 8 cores (4 batch elements per
core); weights replicated. Host-side marshalling casts activations and
weights to bf16 and pre-transposes src/trg to [D, L] so the contraction
dim lands on SBUF partitions with plain contiguous DMAs.

Per-core dataflow (bf16 operands, fp32 PSUM accumulation):
  - sv_T [O-part, L]   = matmul(lhsT=W1, rhs=srcT)  (+ b1 per-partition)
  - tk_T [O-part, N]   = matmul(lhsT=W2, rhs=trgT)  (+ b2 per-partition)
  - tv   [N-part, O]   = matmul(lhsT=trgT, rhs=W1)  (+ b1 via K=1 ones-row matmul)
  - score_T [N-part, L] = matmul(lhsT=tk_T, rhs=sv_T); e = exp(score/32)
    (softmax max-subtraction skipped: scores have std ~0.33, exp is safe)
  - denom over the partition dim via ones-column matmul; r = 1/denom is
    broadcast across partitions with a K=1 fp32 matmul.
  - ctx_T [O-part, L] = matmul(lhsT=tv, rhs=e); hl = sv_T - ctx_T * r
  - h2_T [O-part, L]  = relu(matmul(lhsT=W3a, rhs=[sv_T; hl]) + b3a)
  - out  [L-part, O]  = matmul(lhsT=h2_T, rhs=W3b) (+ b3b via K=1 matmul)
    -> written to DRAM in natural row-major layout (fp32).
"""

import math
from contextlib import ExitStack

import ml_dtypes
import numpy as np

import concourse.bass as bass
import concourse.mybir as mybir
import concourse.tile as tile
from concourse import bacc
from concourse.bass_utils import run_bass_kernel_spmd

P = 128
B_FULL = 32
N_CORES = 8
BS = B_FULL // N_CORES  # 4 batch elements per core
L = 1024
N = 1024
D = 1024
O = 1024

F32 = mybir.dt.float32
BF16 = mybir.dt.bfloat16
AF = mybir.ActivationFunctionType
NP_BF16 = ml_dtypes.bfloat16

LC = 512          # l-chunk size for phase B
N_LC = L // LC    # 2 chunks
KD = D // P       # 8 contraction tiles over D
KO = O // P       # 8 tiles over O
KN = N // P       # 8 tiles over N
K2O = 2 * O // P  # 16 tiles over 2O

INV_SQRT_O = 1.0 / math.sqrt(O)


def _load_weight(nc, dst, w_dram, ktiles):
    """DMA a (ktiles*128, 1024) bf16 weight into [128, ktiles, 1024]."""
    for k in range(ktiles):
        nc.sync.dma_start(dst[:, k, :], w_dram.ap()[k * P:(k + 1) * P, :])


def _load_st(nc, dest, dram, b):
    """DMA a pre-transposed (D, rows) bf16 activation into [128, KD, rows]."""
    for k in range(KD):
        nc.sync.dma_start(dest[:, k, :], dram.ap()[b, k * P:(k + 1) * P, :])


def _build(nc, tc):
    src_t = nc.dram_tensor("srcT", [BS, D, L], BF16, kind="ExternalInput")
    trg_t = nc.dram_tensor("trgT", [BS, D, N], BF16, kind="ExternalInput")
    w1 = nc.dram_tensor("W1bf", [D, O], BF16, kind="ExternalInput")
    w2 = nc.dram_tensor("W2bf", [D, O], BF16, kind="ExternalInput")
    w3a = nc.dram_tensor("W3abf", [2 * O, O], BF16, kind="ExternalInput")
    w3b = nc.dram_tensor("W3bbf", [O, O], BF16, kind="ExternalInput")
    b1 = nc.dram_tensor("b1", [O], F32, kind="ExternalInput")
    b2 = nc.dram_tensor("b2", [O], F32, kind="ExternalInput")
    b3a = nc.dram_tensor("b3a", [O], F32, kind="ExternalInput")
    b1bf = nc.dram_tensor("b1bf", [O], BF16, kind="ExternalInput")
    b3b_f = nc.dram_tensor("b3bf32", [O], F32, kind="ExternalInput")
    out = nc.dram_tensor("out", [BS, L, O], F32, kind="ExternalOutput")

    ctx = ExitStack()
    singles = ctx.enter_context(tc.tile_pool(name="singles", bufs=1))
    stp = ctx.enter_context(tc.tile_pool(name="stp", bufs=2))
    actp = ctx.enter_context(tc.tile_pool(name="actp", bufs=1))
    ehh = ctx.enter_context(tc.tile_pool(name="ehh", bufs=3))
    smallp = ctx.enter_context(tc.tile_pool(name="smallp", bufs=2))
    outp = ctx.enter_context(tc.tile_pool(name="outp", bufs=3))
    psum = ctx.enter_context(tc.tile_pool(name="psum", bufs=6, space="PSUM"))
    auxps = ctx.enter_context(tc.tile_pool(name="auxps", bufs=1, space="PSUM"))

    # ---- constants ----
    w1bf = singles.tile([P, KD, O], BF16)
    w2bf = singles.tile([P, KD, O], BF16)
    w3abf = singles.tile([P, K2O, O], BF16)
    w3bbf = singles.tile([P, KO, O], BF16)
    b1col = singles.tile([P, KO], F32)
    b2col = singles.tile([P, KO], F32)
    b3acol = singles.tile([P, KO], F32)
    b1full = singles.tile([P, O], BF16)    # b1 replicated on all partitions
    b3bfull = singles.tile([P, O], F32)    # b3b replicated on all partitions
    ones_col = singles.tile([P, 1], BF16)
    ones_row_f = singles.tile([1, P], F32)

    # small constants first (they gate the ACT/DVE psum drains)
    nc.sync.dma_start(b1col[:], b1.ap().rearrange("(oo op) -> op oo", op=P))
    nc.sync.dma_start(b2col[:], b2.ap().rearrange("(oo op) -> op oo", op=P))
    nc.sync.dma_start(b3acol[:], b3a.ap().rearrange("(oo op) -> op oo", op=P))
    # bias value rows replicated across all 128 partitions via stride-0 DMA
    nc.sync.dma_start(
        b1full[:], bass.AP(tensor=b1bf.ap().tensor, offset=0, ap=[[0, P], [1, O]]))
    nc.sync.dma_start(
        b3bfull[:], bass.AP(tensor=b3b_f.ap().tensor, offset=0, ap=[[0, P], [1, O]]))
    nc.vector.memset(ones_col[:], 1.0)
    nc.vector.memset(ones_row_f[:], 1.0)
    # W1 + batch-0 activations next: the first matmul groups need only these
    _load_weight(nc, w1bf, w1, KD)
    s_t0 = stp.tile([P, KD, L], BF16, tag="st")
    t_t0 = stp.tile([P, KD, N], BF16, tag="st")
    _load_st(nc, s_t0, src_t, 0)
    _load_st(nc, t_t0, trg_t, 0)
    # remaining weights
    _load_weight(nc, w2bf, w2, KD)
    _load_weight(nc, w3abf, w3a, K2O)
    _load_weight(nc, w3bbf, w3b, KO)

    for b in range(BS):
        if b == 0:
            s_t, t_t = s_t0, t_t0
        else:
            s_t = stp.tile([P, KD, L], BF16, tag="st")
            t_t = stp.tile([P, KD, N], BF16, tag="st")
            _load_st(nc, s_t, src_t, b)
            _load_st(nc, t_t, trg_t, b)

        # ---- phase A: sv_T, tk_T (ACT drain + bias), tv (DVE drain) ----
        svt = actp.tile([P, KO, L], BF16, tag="svt")
        tkt = actp.tile([P, KO, N], BF16, tag="tkt")
        tv = actp.tile([P, KN, O], BF16, tag="tv")
        for j in range(KO):
            for lc in range(N_LC):
                ps = psum.tile([P, LC], F32)
                for k in range(KD):
                    nc.tensor.matmul(
                        ps[:], w1bf[:, k, j * P:(j + 1) * P],
                        s_t[:, k, lc * LC:(lc + 1) * LC],
                        start=(k == 0), stop=(k == KD - 1))
                nc.scalar.activation(
                    svt[:, j, lc * LC:(lc + 1) * LC], ps[:], AF.Identity,
                    bias=b1col[:, j:j + 1])
        for j in range(KO):
            for nch in range(N // LC):
                ps = psum.tile([P, LC], F32)
                for k in range(KD):
                    nc.tensor.matmul(
                        ps[:], w2bf[:, k, j * P:(j + 1) * P],
                        t_t[:, k, nch * LC:(nch + 1) * LC],
                        start=(k == 0), stop=(k == KD - 1))
                nc.scalar.activation(
                    tkt[:, j, nch * LC:(nch + 1) * LC], ps[:], AF.Identity,
                    bias=b2col[:, j:j + 1])
        for i in range(KN):
            for oc in range(O // LC):
                ps = psum.tile([P, LC], F32)
                for k in range(KD):
                    nc.tensor.matmul(
                        ps[:], t_t[:, k, i * P:(i + 1) * P],
                        w1bf[:, k, oc * LC:(oc + 1) * LC],
                        start=(k == 0), stop=(k == KD - 1))
                nc.vector.tensor_add(tv[:, i, oc * LC:(oc + 1) * LC], ps[:],
                                     b1full[:, oc * LC:(oc + 1) * LC])

        # ---- phase B: per l-chunk ----
        for lc in range(N_LC):
            lsl = slice(lc * LC, (lc + 1) * LC)
            # score_T -> e = exp(score / sqrt(O)), [n-part, l]
            e = ehh.tile([P, KN, LC], BF16, tag="ehh")
            for i in range(KN):
                ps = psum.tile([P, LC], F32)
                for k in range(KO):
                    nc.tensor.matmul(
                        ps[:], tkt[:, k, i * P:(i + 1) * P], svt[:, k, lsl],
                        start=(k == 0), stop=(k == KO - 1))
                nc.scalar.activation(e[:, i, :], ps[:], AF.Exp, scale=INV_SQRT_O)
            # denominator: sum e over n (partitions) via ones-column matmul
            d_ps = auxps.tile([1, LC], F32, tag="dps")
            for i in range(KN):
                nc.tensor.matmul(d_ps[:], ones_col[:, :1], e[:, i, :],
                                 start=(i == 0), stop=(i == KN - 1))
            r_sb = smallp.tile([1, LC], F32, tag="rsb", bufs=1)
            nc.vector.reciprocal(r_sb[:], d_ps[:])
            # broadcast r over 128 partitions with a K=1 fp32 matmul
            r_ps = auxps.tile([P, LC], F32, tag="rps")
            nc.tensor.matmul(r_ps[:], ones_row_f[:1, :], r_sb[:1, :],
                             start=True, stop=True)
            rbc = smallp.tile([P, LC], F32, tag="rbc", bufs=1)
            nc.vector.tensor_copy(rbc[:], r_ps[:])
            # ctx_T + normalize + hl = sv_T - ctx_T/denom
            hl = ehh.tile([P, KO, LC], BF16, tag="ehh")
            for j in range(KO):
                ps = psum.tile([P, LC], F32)
                for i in range(KN):
                    nc.tensor.matmul(
                        ps[:], tv[:, i, j * P:(j + 1) * P], e[:, i, :],
                        start=(i == 0), stop=(i == KN - 1))
                ctxn = smallp.tile([P, LC], F32, tag="ctxn", bufs=1)
                nc.vector.tensor_mul(ctxn[:], ps[:], rbc[:])
                nc.vector.tensor_sub(hl[:, j, :], svt[:, j, lsl], ctxn[:])
            # fc3a: h2 = relu([sv_T; hl] contracted with W3a + b3a)
            h2 = ehh.tile([P, KO, LC], BF16, tag="ehh")
            for j2 in range(KO):
                ps = psum.tile([P, LC], F32)
                for k in range(K2O):
                    rhs = svt[:, k, lsl] if k < KO else hl[:, k - KO, :]
                    nc.tensor.matmul(
                        ps[:], w3abf[:, k, j2 * P:(j2 + 1) * P], rhs,
                        start=(k == 0), stop=(k == K2O - 1))
                nc.scalar.activation(h2[:, j2, :], ps[:], AF.Relu,
                                     bias=b3acol[:, j2:j2 + 1])
            # fc3b: out natural [l-part, o] + b3b via K=1 matmul
            for lt in range(LC // P):
                for oc in range(O // LC):
                    ps = psum.tile([P, LC], F32)
                    for k in range(KO):
                        nc.tensor.matmul(
                            ps[:], h2[:, k, lt * P:(lt + 1) * P],
                            w3bbf[:, k, oc * LC:(oc + 1) * LC],
                            start=(k == 0), stop=(k == KO - 1))
                    o_sb = outp.tile([P, LC], F32, tag="osb")
                    nc.vector.tensor_add(o_sb[:], ps[:],
                                         b3bfull[:, oc * LC:(oc + 1) * LC])
                    nc.sync.dma_start(
                        out.ap()[b, lc * LC + lt * P: lc * LC + (lt + 1) * P,
                                 oc * LC:(oc + 1) * LC],
                        o_sb[:])

    ctx.close()


_NC_CACHE = None


def _get_nc():
    global _NC_CACHE
    if _NC_CACHE is None:
        nc = bacc.Bacc("TRN2", target_bir_lowering=False, debug=False,
                       num_devices=N_CORES)
        with tile.TileContext(nc) as tc:
            _build(nc, tc)
        nc.compile()
        _NC_CACHE = nc
    return _NC_CACHE


def kernel(**inputs):
    nc = _get_nc()
    src = np.asarray(inputs["src"], dtype=np.float32)
    trg = np.asarray(inputs["trg"], dtype=np.float32)
    # host-side marshalling: bf16 cast + transpose so the contraction dim
    # (D) lands on SBUF partitions with contiguous DMAs on-device.
    src_t = np.ascontiguousarray(
        src.astype(NP_BF16).transpose(0, 2, 1))   # (B, D, L)
    trg_t = np.ascontiguousarray(
        trg.astype(NP_BF16).transpose(0, 2, 1))   # (B, D, N)
    shared = {
        "W1bf": np.ascontiguousarray(np.asarray(inputs["W1"], np.float32).astype(NP_BF16)),
        "W2bf": np.ascontiguousarray(np.asarray(inputs["W2"], np.float32).astype(NP_BF16)),
        "W3abf": np.ascontiguousarray(np.asarray(inputs["W3a"], np.float32).astype(NP_BF16)),
        "W3bbf": np.ascontiguousarray(np.asarray(inputs["W3b"], np.float32).astype(NP_BF16)),
        "b1": np.ascontiguousarray(np.asarray(inputs["b1"], np.float32)),
        "b2": np.ascontiguousarray(np.asarray(inputs["b2"], np.float32)),
        "b3a": np.ascontiguousarray(np.asarray(inputs["b3a"], np.float32)),
        "b1bf": np.ascontiguousarray(np.asarray(inputs["b1"], np.float32).astype(NP_BF16)),
        "b3bf32": np.ascontiguousarray(np.asarray(inputs["b3b"], np.float32)),
    }
    in_maps = []
    for c in range(N_CORES):
        m = dict(shared)
        m["srcT"] = src_t[c * BS:(c + 1) * BS]
        m["trgT"] = trg_t[c * BS:(c + 1) * BS]
        in_maps.append(m)
    res = run_bass_kernel_spmd(nc, in_maps, core_ids=list(range(N_CORES)))
    return np.concatenate([r["out"] for r in res.results], axis=0)


# revision 10
# speedup vs baseline: 1.3445x; 1.0223x over previous
"""Trainium2 Bass kernel for nn_DiffModule_40827959116531 (sparse_attention).

Reference computation (per batch element b):
    sv  = src @ W1 + b1                      # (L, O)
    tk  = trg @ W2 + b2                      # (N, O)
    tv  = trg @ W1 + b1                      # (N, O)
    score = sv @ tk.T / sqrt(O)              # (L, N)
    prob  = softmax(score, axis=-1)
    ctx   = prob @ tv                        # (L, O)
    h   = concat([sv, sv - ctx], -1)         # (L, 2O)
    h2  = relu(h @ W3a + b3a)                # (L, O)
    out = h2 @ W3b + b3b                     # (L, O)

Sharding: data-parallel over B=32 across 8 cores (4 batch elements per
core); weights replicated. Host-side marshalling casts activations and
weights to bf16 and pre-transposes src/trg to [D, L] so the contraction
dim lands on SBUF partitions with plain contiguous DMAs.

Per-core dataflow (bf16 operands, fp32 PSUM accumulation):
  - sv_T [O-part, L]   = matmul(lhsT=W1, rhs=srcT)  (+ b1 per-partition)
  - tk_T [O-part, N]   = matmul(lhsT=W2, rhs=trgT)  (+ b2 per-partition)
  - tv   [N-part, O]   = matmul(lhsT=trgT, rhs=W1)  (+ b1 via K=1 ones-row matmul)
  - score_T [N-part, L] = matmul(lhsT=tk_T, rhs=sv_T); e = exp(score/32)
    (softmax max-subtraction skipped: scores have std ~0.33, exp is safe)
  - denom over the partition dim via ones-column matmul; r = 1/denom is
    broadcast across partitions with a K=1 fp32 matmul.
  - ctx_T [O-part, L] = matmul(lhsT=tv, rhs=e); hl = sv_T - ctx_T * r
  - h2_T [O-part, L]  = relu(matmul(lhsT=W3a, rhs=[sv_T; hl]) + b3a)
  - out  [L-part, O]  = matmul(lhsT=h2_T, rhs=W3b) (+ b3b via K=1 matmul)
    -> written to DRAM in natural row-major layout (fp32).
"""

import math
from contextlib import ExitStack

import ml_dtypes
import numpy as np

import concourse.bass as bass
import concourse.mybir as mybir
import concourse.tile as tile
from concourse import bacc
from concourse.bass_utils import run_bass_kernel_spmd

P = 128
B_FULL = 32
N_CORES = 8
BS = B_FULL // N_CORES  # 4 batch elements per core
L = 1024
N = 1024
D = 1024
O = 1024

F32 = mybir.dt.float32
BF16 = mybir.dt.bfloat16
AF = mybir.ActivationFunctionType
NP_BF16 = ml_dtypes.bfloat16

LC = 512          # l-chunk size for phase B
N_LC = L // LC    # 2 chunks
KD = D // P       # 8 contraction tiles over D
KO = O // P       # 8 tiles over O
KN = N // P       # 8 tiles over N
K2O = 2 * O // P  # 16 tiles over 2O

INV_SQRT_O = 1.0 / math.sqrt(O)


def _load_weight(nc, dst, w_dram, ktiles):
    """DMA a (ktiles*128, 1024) bf16 weight into [128, ktiles, 1024]."""
    for k in range(ktiles):
        nc.sync.dma_start(dst[:, k, :], w_dram.ap()[k * P:(k + 1) * P, :])


def _load_st(nc, dest, dram, b):
    """DMA a pre-transposed (D, rows) bf16 activation into [128, KD, rows]."""
    for k in range(KD):
        nc.sync.dma_start(dest[:, k, :], dram.ap()[b, k * P:(k + 1) * P, :])


def _build(nc, tc):
    src_t = nc.dram_tensor("srcT", [BS, D, L], BF16, kind="ExternalInput")
    trg_t = nc.dram_tensor("trgT", [BS, D, N], BF16, kind="ExternalInput")
    w1 = nc.dram_tensor("W1bf", [D, O], BF16, kind="ExternalInput")
    w2 = nc.dram_tensor("W2bf", [D, O], BF16, kind="ExternalInput")
    w3a = nc.dram_tensor("W3abf", [2 * O, O], BF16, kind="ExternalInput")
    w3b = nc.dram_tensor("W3bbf", [O, O], BF16, kind="ExternalInput")
    b1 = nc.dram_tensor("b1", [O], F32, kind="ExternalInput")
    b2 = nc.dram_tensor("b2", [O], F32, kind="ExternalInput")
    b3a = nc.dram_tensor("b3a", [O], F32, kind="ExternalInput")
    b1bf = nc.dram_tensor("b1bf", [O], BF16, kind="ExternalInput")
    b3b_f = nc.dram_tensor("b3bf32", [O], F32, kind="ExternalInput")
    out = nc.dram_tensor("out", [BS, L, O], F32, kind="ExternalOutput")

    ctx = ExitStack()
    singles = ctx.enter_context(tc.tile_pool(name="singles", bufs=1))
    stp = ctx.enter_context(tc.tile_pool(name="stp", bufs=2))
    actp = ctx.enter_context(tc.tile_pool(name="actp", bufs=1))
    ehh = ctx.enter_context(tc.tile_pool(name="ehh", bufs=3))
    smallp = ctx.enter_context(tc.tile_pool(name="smallp", bufs=2))
    outp = ctx.enter_context(tc.tile_pool(name="outp", bufs=3))
    psum = ctx.enter_context(tc.tile_pool(name="psum", bufs=6, space="PSUM"))
    auxps = ctx.enter_context(tc.tile_pool(name="auxps", bufs=1, space="PSUM"))

    # ---- constants ----
    w1bf = singles.tile([P, KD, O], BF16)
    w2bf = singles.tile([P, KD, O], BF16)
    w3abf = singles.tile([P, K2O, O], BF16)
    w3bbf = singles.tile([P, KO, O], BF16)
    b1col = singles.tile([P, KO], F32)
    b2col = singles.tile([P, KO], F32)
    b3acol = singles.tile([P, KO], F32)
    b1full = singles.tile([P, O], BF16)    # b1 replicated on all partitions
    b3bfull = singles.tile([P, O], F32)    # b3b replicated on all partitions
    ones_col = singles.tile([P, 1], BF16)
    ones_row_f = singles.tile([1, P], F32)

    # small constants first (they gate the ACT/DVE psum drains)
    nc.sync.dma_start(b1col[:], b1.ap().rearrange("(oo op) -> op oo", op=P))
    nc.sync.dma_start(b2col[:], b2.ap().rearrange("(oo op) -> op oo", op=P))
    nc.sync.dma_start(b3acol[:], b3a.ap().rearrange("(oo op) -> op oo", op=P))
    # bias value rows replicated across all 128 partitions via stride-0 DMA
    nc.sync.dma_start(
        b1full[:], bass.AP(tensor=b1bf.ap().tensor, offset=0, ap=[[0, P], [1, O]]))
    nc.sync.dma_start(
        b3bfull[:], bass.AP(tensor=b3b_f.ap().tensor, offset=0, ap=[[0, P], [1, O]]))
    nc.vector.memset(ones_col[:], 1.0)
    nc.vector.memset(ones_row_f[:], 1.0)
    # W1 + batch-0 activations next: the first matmul groups need only these
    s_t0 = stp.tile([P, KD, L], BF16, tag="st")
    t_t0 = stp.tile([P, KD, N], BF16, tag="st")
    for k in range(KD):
        nc.sync.dma_start(w1bf[:, k, :], w1.ap()[k * P:(k + 1) * P, :])
        nc.sync.dma_start(s_t0[:, k, :], src_t.ap()[0, k * P:(k + 1) * P, :])
    _load_st(nc, t_t0, trg_t, 0)
    # remaining weights
    _load_weight(nc, w2bf, w2, KD)
    _load_weight(nc, w3abf, w3a, K2O)
    _load_weight(nc, w3bbf, w3b, KO)

    for b in range(BS):
        if b == 0:
            s_t, t_t = s_t0, t_t0
        else:
            s_t = stp.tile([P, KD, L], BF16, tag="st")
            t_t = stp.tile([P, KD, N], BF16, tag="st")
            _load_st(nc, s_t, src_t, b)
            _load_st(nc, t_t, trg_t, b)

        # ---- phase A: sv_T, tk_T (ACT drain + bias), tv (DVE drain) ----
        svt = actp.tile([P, KO, L], BF16, tag="svt")
        tkt = actp.tile([P, KO, N], BF16, tag="tkt")
        tv = actp.tile([P, KN, O], BF16, tag="tv")
        for j in range(KO):
            for lc in range(N_LC):
                ps = psum.tile([P, LC], F32)
                for k in range(KD):
                    nc.tensor.matmul(
                        ps[:], w1bf[:, k, j * P:(j + 1) * P],
                        s_t[:, k, lc * LC:(lc + 1) * LC],
                        start=(k == 0), stop=(k == KD - 1))
                nc.scalar.activation(
                    svt[:, j, lc * LC:(lc + 1) * LC], ps[:], AF.Identity,
                    bias=b1col[:, j:j + 1])
        for j in range(KO):
            for nch in range(N // LC):
                ps = psum.tile([P, LC], F32)
                for k in range(KD):
                    nc.tensor.matmul(
                        ps[:], w2bf[:, k, j * P:(j + 1) * P],
                        t_t[:, k, nch * LC:(nch + 1) * LC],
                        start=(k == 0), stop=(k == KD - 1))
                nc.scalar.activation(
                    tkt[:, j, nch * LC:(nch + 1) * LC], ps[:], AF.Identity,
                    bias=b2col[:, j:j + 1])
        for i in range(KN):
            for oc in range(O // LC):
                ps = psum.tile([P, LC], F32)
                for k in range(KD):
                    nc.tensor.matmul(
                        ps[:], t_t[:, k, i * P:(i + 1) * P],
                        w1bf[:, k, oc * LC:(oc + 1) * LC],
                        start=(k == 0), stop=(k == KD - 1))
                nc.vector.tensor_add(tv[:, i, oc * LC:(oc + 1) * LC], ps[:],
                                     b1full[:, oc * LC:(oc + 1) * LC])

        # ---- phase B: per l-chunk ----
        for lc in range(N_LC):
            lsl = slice(lc * LC, (lc + 1) * LC)
            # score_T -> e = exp(score / sqrt(O)); denominator matmuls
            # (sum over partitions via ones column) interleave per i-tile
            e = ehh.tile([P, KN, LC], BF16, tag="ehh")
            d_ps = auxps.tile([1, LC], F32, tag="dps")
            for i in range(KN):
                ps = psum.tile([P, LC], F32)
                for k in range(KO):
                    nc.tensor.matmul(
                        ps[:], tkt[:, k, i * P:(i + 1) * P], svt[:, k, lsl],
                        start=(k == 0), stop=(k == KO - 1))
                nc.scalar.activation(e[:, i, :], ps[:], AF.Exp, scale=INV_SQRT_O)
                nc.tensor.matmul(d_ps[:], ones_col[:, :1], e[:, i, :],
                                 start=(i == 0), stop=(i == KN - 1))
            # ctx_T + normalize + hl = sv_T - ctx_T/denom. The reciprocal +
            # partition-broadcast (K=1 fp32 matmul) are emitted after two ctx
            # groups so the PE has work while the DVE computes 1/denom.
            hl = ehh.tile([P, KO, LC], BF16, tag="ehh")
            ctx_ps = []
            r_sb = None
            for j in range(KO):
                ps = psum.tile([P, LC], F32)
                for i in range(KN):
                    nc.tensor.matmul(
                        ps[:], tv[:, i, j * P:(j + 1) * P], e[:, i, :],
                        start=(i == 0), stop=(i == KN - 1))
                ctx_ps.append(ps)
                if j == 1:
                    r_sb = smallp.tile([1, LC], F32, tag="rsb", bufs=1)
                    nc.vector.reciprocal(r_sb[:], d_ps[:])
                    r_ps = auxps.tile([P, LC], F32, tag="rps")
                    nc.tensor.matmul(r_ps[:], ones_row_f[:1, :], r_sb[:1, :],
                                     start=True, stop=True)
                    rbc = smallp.tile([P, LC], F32, tag="rbc", bufs=1)
                    nc.vector.tensor_copy(rbc[:], r_ps[:])
                    for jj in range(2):
                        ctxn = smallp.tile([P, LC], F32, tag="ctxn", bufs=2)
                        nc.vector.tensor_mul(ctxn[:], ctx_ps[jj][:], rbc[:])
                        nc.vector.tensor_sub(hl[:, jj, :], svt[:, jj, lsl], ctxn[:])
                elif j > 1:
                    ctxn = smallp.tile([P, LC], F32, tag="ctxn", bufs=2)
                    nc.vector.tensor_mul(ctxn[:], ps[:], rbc[:])
                    nc.vector.tensor_sub(hl[:, j, :], svt[:, j, lsl], ctxn[:])
            # fc3a: h2 = relu([sv_T; hl] contracted with W3a + b3a)
            h2 = ehh.tile([P, KO, LC], BF16, tag="ehh")
            for j2 in range(KO):
                ps = psum.tile([P, LC], F32)
                for k in range(K2O):
                    rhs = svt[:, k, lsl] if k < KO else hl[:, k - KO, :]
                    nc.tensor.matmul(
                        ps[:], w3abf[:, k, j2 * P:(j2 + 1) * P], rhs,
                        start=(k == 0), stop=(k == K2O - 1))
                nc.scalar.activation(h2[:, j2, :], ps[:], AF.Relu,
                                     bias=b3acol[:, j2:j2 + 1])
            # fc3b: out natural [l-part, o] + b3b via K=1 matmul
            for lt in range(LC // P):
                for oc in range(O // LC):
                    ps = psum.tile([P, LC], F32)
                    for k in range(KO):
                        nc.tensor.matmul(
                            ps[:], h2[:, k, lt * P:(lt + 1) * P],
                            w3bbf[:, k, oc * LC:(oc + 1) * LC],
                            start=(k == 0), stop=(k == KO - 1))
                    o_sb = outp.tile([P, LC], F32, tag="osb")
                    nc.vector.tensor_add(o_sb[:], ps[:],
                                         b3bfull[:, oc * LC:(oc + 1) * LC])
                    nc.sync.dma_start(
                        out.ap()[b, lc * LC + lt * P: lc * LC + (lt + 1) * P,
                                 oc * LC:(oc + 1) * LC],
                        o_sb[:])

    ctx.close()


_NC_CACHE = None


def _get_nc():
    global _NC_CACHE
    if _NC_CACHE is None:
        nc = bacc.Bacc("TRN2", target_bir_lowering=False, debug=False,
                       num_devices=N_CORES)
        with tile.TileContext(nc) as tc:
            _build(nc, tc)
        nc.compile()
        _NC_CACHE = nc
    return _NC_CACHE


def kernel(**inputs):
    nc = _get_nc()
    src = np.asarray(inputs["src"], dtype=np.float32)
    trg = np.asarray(inputs["trg"], dtype=np.float32)
    # host-side marshalling: bf16 cast + transpose so the contraction dim
    # (D) lands on SBUF partitions with contiguous DMAs on-device.
    src_t = np.ascontiguousarray(
        src.astype(NP_BF16).transpose(0, 2, 1))   # (B, D, L)
    trg_t = np.ascontiguousarray(
        trg.astype(NP_BF16).transpose(0, 2, 1))   # (B, D, N)
    shared = {
        "W1bf": np.ascontiguousarray(np.asarray(inputs["W1"], np.float32).astype(NP_BF16)),
        "W2bf": np.ascontiguousarray(np.asarray(inputs["W2"], np.float32).astype(NP_BF16)),
        "W3abf": np.ascontiguousarray(np.asarray(inputs["W3a"], np.float32).astype(NP_BF16)),
        "W3bbf": np.ascontiguousarray(np.asarray(inputs["W3b"], np.float32).astype(NP_BF16)),
        "b1": np.ascontiguousarray(np.asarray(inputs["b1"], np.float32)),
        "b2": np.ascontiguousarray(np.asarray(inputs["b2"], np.float32)),
        "b3a": np.ascontiguousarray(np.asarray(inputs["b3a"], np.float32)),
        "b1bf": np.ascontiguousarray(np.asarray(inputs["b1"], np.float32).astype(NP_BF16)),
        "b3bf32": np.ascontiguousarray(np.asarray(inputs["b3b"], np.float32)),
    }
    in_maps = []
    for c in range(N_CORES):
        m = dict(shared)
        m["srcT"] = src_t[c * BS:(c + 1) * BS]
        m["trgT"] = trg_t[c * BS:(c + 1) * BS]
        in_maps.append(m)
    res = run_bass_kernel_spmd(nc, in_maps, core_ids=list(range(N_CORES)))
    return np.concatenate([r["out"] for r in res.results], axis=0)
